# revision 34
# baseline (speedup 1.0000x reference)
"""Multi-head attention (B=4, S=2048, E=768, H=12) on 8 trn2 NeuronCores.

Sharding: tensor-parallel over heads x data-parallel over batch. Core c
handles batch b=c//2 and heads 6*(c%2)..6*(c%2)+5 (all 2048 queries). Each
core emits a partial output projection (its 6 heads' contribution); the two
cores of a batch pair are summed on the host during unsharding. The bias is
added on device by the even core only (odd cores receive a zero bias).

Layouts: matmul operands keep "feature on partitions" so that
  - the qkv projection emits Q^T/K^T directly (lhsT=w^T chunk, rhs=x^T chunk),
  - Q@K^T emits S^T = [k, q] (lhsT=K^T slice, rhs=Q^T slice, contract d=64),
  - softmax row sums come from a ones-column appended to V (AV matmul M=65),
  - attention output lands as outT [e', q] - exactly the lhsT the output
    projection wants.

x arrives fp16 (host-converted, halves the input DMA) in four 512-row
quad loads on the SP queue while the weight loads ride the Activation
queue; x^T tiles come from fp16 PE transposes (1 cyc/row vs fp32's 2)
whose PSUM strips are evacuated by ScalarE (idle during stage A).

Softmax exp runs on BOTH ScalarE and the DVE, alternating by [128,512]
half-chunk (3:2 pattern):
  - 3/5 of chunks: ScalarE table exp over PSUM strips, 1/sqrt(d) folded
    into the activation scale,
  - 2/5 of chunks: DVE Schraudolph - es = bitcast_fp16(int16(l*(2^10*log2
    e)/8 + 15360 - 59 + 0.5)), one tensor_scalar (mult,add) per strip.
    ~2% rms per-weight error on those chunks => ~1.2e-2 end-to-end rel err
    (<2e-2 gate).
This halves the ScalarE exp time that otherwise paces the attention phase
(ScalarE 1.2 GHz x 128 lanes = 164us of exp vs the PE's 164us of matmul).
Max-subtraction is skipped (logits ~N(0,1), exp cannot overflow; Schraudolph
int16 is monotone-safe for |logit|<11).

Stage-A PSUM evacuations run on ScalarE (idle there), keeping the DVE free
for attention. Output is stored fp16 (partial sums; host accumulates fp32).

Dtypes: all matmul operands are fp16 (1 cyc/row like bf16 but 4x the
mantissa; every intermediate here is well inside fp16 range) with fp32 PSUM
accumulation. Softmax normalization (sums-row copy, reciprocal, DRAM-bounce
partition broadcast of 1/sum, multiply) is emitted DEFERRED - interleaved
into the next (h, half) block's iterations - so the in-order DVE queue
never blocks a Schraudolph exp behind a normalize dependency (a
normalize-blocked DVE exp stalls the PE >1us and trips the HAM clock gate
to half frequency for the rest of the kernel).

Environment workaround (this walrus build): sync-waits are split one
per instruction onto NoOps (_split_waits, _TC).
"""

import numpy as np

import concourse.bass as bass
import concourse.tile as tile
from concourse import mybir
from concourse.bass_utils import run_bass_kernel_spmd
from concourse.tile import ScopedClock

B, S, E, H, D = 4, 2048, 768, 12, 64
NCORES = 8
HL = 6               # heads per core
FL = HL * D          # 384 local feature dim
SCALE = D ** -0.5
FP = mybir.dt.float32
F16 = mybir.dt.float16
I16 = mybir.dt.int16
P = 128

ET = E // P          # 6 e-chunks of 128
FT = FL // P         # 3 local f-tiles of 128
NKT = S // P         # 16 k-tiles of 128
NQC = S // 512       # 4 q-chunks of 512
NST = S // P         # 16 s-tiles
DV = D + 1           # 65: V plus ones column

# Schraudolph exp in fp16 bit-space: round(l * 2^10*log2(e)*SCALE + B).
# B = 15360 (fp16 exponent bias<<10) - 59 (min-RMS centering) + 0.5 (the
# engine's float->int conversion truncates; +0.5 makes it round-to-nearest).
SCHRAUD_A = float(1024.0 * np.log2(np.e) * SCALE)
SCHRAUD_B = 15360.0 - 59.0 + 0.5


class _TC(tile.TileContext):
    """TileContext with the end-of-kernel drain's sem waits split one per
    instruction (this walrus build's CTRL_NO_STRUCT encoding holds only one
    sync wait; the stock drain carries one wait per outstanding proc)."""

    def _drain_and_barrier(self, tick_clock, wait_clock):
        probe = self.nc.sync.nop()
        wait_clock.add_sem_waits(
            probe.ins, ScopedClock({None: tick_clock.global_clock})
        )
        si = probe.ins.sync_info
        waits = list(si.on_wait) if si is not None else []
        if len(waits) > 1:
            si.on_wait = waits[:1]
            for w in waits[1:]:
                n = self.nc.sync.nop()
                n.ins.sync_info = type(si)(on_wait=[w], on_update=[])
        self.nc.sync.drain()
        self.nc.all_engine_barrier()
        popped = self.nc._tile_sem_poison_stack.pop()
        assert popped is self._sem_poison
        self.nc.clear_and_free_semaphores(list(self.sems.allocated().values()))
        self.nc.all_engine_barrier()


def _split_waits(nc):
    """This walrus build accepts at most one sync-wait per TPB instruction
    (two on EventSemaphore). Tile emits up to 2-3. Hoist the extras onto
    same-engine NoOps inserted immediately before the instruction."""
    ctr = [0]
    for f in nc.m.functions:
        for bb in f.blocks:
            out = []
            changed = False
            for inst in bb.instructions:
                si = getattr(inst, "sync_info", None)
                if si is not None and si.on_wait:
                    cap = 2 if isinstance(inst, mybir.InstEventSemaphore) else 1
                    waits = list(si.on_wait)
                    if len(waits) > cap:
                        changed = True
                        for w in waits[:-cap]:
                            ctr[0] += 1
                            out.append(
                                mybir.InstNoOp(
                                    name=f"WSPLIT-{ctr[0]}",
                                    engine=inst.engine,
                                    ins=[],
                                    outs=[],
                                    sync_info=mybir.SyncInfo(
                                        on_wait=[w], on_update=[]
                                    ),
                                    bass_nofuse=True,
                                )
                            )
                        si.on_wait = waits[-cap:]
                        inst.sync_info = si
                out.append(inst)
            if changed:
                bb.instructions = out


def build(n_reps=1):
    nc = bass.Bass()
    xb = nc.dram_tensor("xb", [S, E], F16, kind="ExternalInput")
    wqkvT = nc.dram_tensor("wqkvT", [E, 3 * FL], F16, kind="ExternalInput")
    wprojT = nc.dram_tensor("wprojT", [FL, E], F16, kind="ExternalInput")
    identd = nc.dram_tensor("identd", [P, P], F16, kind="ExternalInput")
    out = nc.dram_tensor("out", [S, E], F16, kind="ExternalOutput")

    Exp = mybir.ActivationFunctionType.Exp
    Mult = mybir.AluOpType.mult
    Add = mybir.AluOpType.add

    from contextlib import ExitStack

    with _TC(nc) as tc, ExitStack() as stack:
        consts = stack.enter_context(tc.tile_pool(name="consts", bufs=1))
        persist = stack.enter_context(tc.tile_pool(name="persist", bufs=1))

        ident = consts.tile([P, P], F16)
        nc.sync.dma_start(ident[:], identd[:])
        warm_exp = consts.tile([1, 8], F16)

        wproj_sb = [
            consts.tile([P, E], F16, tag=f"wproj{c}", name=f"wproj{c}")
            for c in range(FT)
        ]

        # persistent activations
        qT = [persist.tile([P, S], F16, tag=f"qT{t}", name=f"qT{t}") for t in range(FT)]
        kT = [persist.tile([P, S], F16, tag=f"kT{t}", name=f"kT{t}") for t in range(FT)]
        vp = [persist.tile([P, HL * DV], F16, tag=f"vp{t}", name=f"vp{t}") for t in range(NST)]
        outT = [persist.tile([P, S], F16, tag=f"outT{t}", name=f"outT{t}") for t in range(FT)]

        for _rep in range(n_reps):
            # ---------------- Stage A: x^T PE transposes + projections ------
            with tc.tile_pool(name="stagea", bufs=1) as stagea:

                wqkv_sb = [
                    stagea.tile([P, 3 * FL], F16, tag=f"wqkv{c}", name=f"wqkv{c}")
                    for c in range(ET)
                ]
                xbT = [
                    stagea.tile([P, S], F16, tag=f"xbT{c}", name=f"xbT{c}")
                    for c in range(ET)
                ]

                # x quads alternate between the SP and ACT hwdge queues so the
                # transpose feed runs at 2x one queue's rate; weights follow
                # on both queues.
                xq_tiles = []
                with tc.tile_pool(name="xload", bufs=4) as xload, \
                     tc.tile_pool(name="tr_psum", bufs=6, space="PSUM") as tr_psum:
                    for tq in range(4):
                        xq = xload.tile([P, 4 * E], F16, tag="xq")
                        eng = nc.sync if tq % 2 == 0 else nc.scalar
                        eng.dma_start(
                            xq.rearrange("p (k e) -> p k e", e=E),
                            xb[512 * tq : 512 * (tq + 1), :].rearrange(
                                "(k p) e -> p k e", p=P
                            ),
                        )
                        xq_tiles.append(xq)
                    for c in range(ET):
                        eng = nc.sync if c % 2 == 0 else nc.scalar
                        eng.dma_start(
                            wqkv_sb[c][:], wqkvT[P * c : P * (c + 1), :]
                        )
                    for c in range(FT):
                        eng = nc.sync if c % 2 == 0 else nc.scalar
                        eng.dma_start(
                            wproj_sb[c][:], wprojT[P * c : P * (c + 1), :]
                        )
                    # preload the ACT exp table so stage B's first activation
                    # doesn't stall the pipe on a 1.3us ACT_TABLE_LOAD
                    nc.scalar.activation(warm_exp[:], ident[0:1, 0:8], Exp, scale=1.0)

                    for tq in range(4):
                        xq = xq_tiles[tq]
                        for c in range(ET):
                            pt = tr_psum.tile([P, 512], F16, tag="tr")
                            for dt in range(4):
                                nc.tensor.transpose(
                                    pt[:, P * dt : P * (dt + 1)],
                                    xq[:, E * dt + P * c : E * dt + P * (c + 1)],
                                    ident[:],
                                )
                            # evacuations alternate ACT/DVE so the last
                            # quad's strips drain 2x as fast (the qkproj
                            # psum pool reuses these banks)
                            if c % 2 == 0:
                                nc.scalar.copy(
                                    xbT[c][:, 512 * tq : 512 * (tq + 1)], pt[:]
                                )
                            else:
                                nc.vector.tensor_copy(
                                    xbT[c][:, 512 * tq : 512 * (tq + 1)], pt[:]
                                )

                with tc.tile_pool(name="mm_psum", bufs=4, space="PSUM") as mm_psum, \
                     tc.tile_pool(name="v_psum", bufs=3, space="PSUM") as v_psum:
                    # Q^T [384, S] then K^T [384, S]
                    for j in range(NQC):
                        for which, dst in ((0, qT), (1, kT)):
                            for ft in range(FT):
                                pq = mm_psum.tile([P, 512], FP, tag="mm")
                                for c in range(ET):
                                    nc.tensor.matmul(
                                        pq[:],
                                        (wqkv_sb[c][:, FL * which + P * ft : FL * which + P * (ft + 1)]),
                                        (xbT[c][:, 512 * j : 512 * (j + 1)]),
                                        start=(c == 0),
                                        stop=(c == ET - 1),
                                    )
                                nc.scalar.copy(dst[ft][:, 512 * j : 512 * (j + 1)], pq[:])

                    # V [S, 384] natural layout + interleaved ones columns
                    for t in range(NST):
                        pv = v_psum.tile([P, FL], FP, tag="v")
                        for c in range(ET):
                            nc.tensor.matmul(
                                pv[:],
                                (xbT[c][:, P * t : P * (t + 1)]),
                                (wqkv_sb[c][:, 2 * FL : 3 * FL]),
                                start=(c == 0),
                                stop=(c == ET - 1),
                            )
                        v3 = vp[t].rearrange("p (h d) -> p h d", d=DV)
                        nc.scalar.copy(
                            v3[:, :, 0:D], pv[:].rearrange("p (h d) -> p h d", d=D)
                        )
                        nc.vector.memset(v3[:, :, D : D + 1], 1.0)

            # ---------------- Stage B: attention, half-sequence strips ------
            HQ = S // 2  # 1024 queries per strip
            eidx = 0     # exp instruction counter for ACT/DVE alternation
            # Scores/exp run in [128,512] half-chunks: the s_psum ring is four
            # single-bank tiles (2 iterations of lookahead) and each chunk's
            # exp is a ~600ns op, so the scores->exp->AV latency cycle
            # amortizes to ~650ns/iter and the PE stream paces the phase.
            # o_psum stays 4 (two blocks of po slack) so the deferred
            # normalize is never on the PE's critical path.
            with tc.tile_pool(name="s_psum", bufs=4, space="PSUM") as s_psum, \
                 tc.tile_pool(name="o_psum", bufs=4, space="PSUM") as o_psum, \
                 tc.tile_pool(name="expst", bufs=8) as expst, \
                 tc.tile_pool(name="smalls", bufs=6) as smalls, \
                 tc.tile_pool(name="invdram", bufs=6, space="DRAM") as invdram:

                # Softmax normalize (sums row -> DRAM -> [128,4] so the DVE
                # reciprocal runs 128-wide; a [1,512] InstReciprocal costs
                # 3.3us! -> DRAM -> [64,512] partition broadcast -> multiply)
                # is pipelined one block behind and emitted in THREE phases
                # interleaved into the next (h, half) block's iterations, so
                # every DVE op executes with its inputs already resident. The
                # DVE queue is in-order: one DVE op stuck waiting on a DMA
                # delays the Schraudolph exps behind it, stalls the PE >1us,
                # and trips the HAM clock gate to half frequency.
                def norm_phase1a(pend):
                    po, ht, hb, q0 = pend
                    s4s = []
                    for j in range(2):
                        srow = smalls.tile([1, 512], FP, tag="srow")
                        nc.vector.tensor_copy(srow[:], po[j][D : D + 1, :])
                        sd = invdram.tile([1, 512], FP, tag="sd")
                        nc.gpsimd.dma_start(sd[:], srow[:])
                        s4 = smalls.tile([P, 4], FP, tag="s4")
                        nc.gpsimd.dma_start(
                            s4[:], sd.rearrange("a (p f) -> (a p) f", p=P)
                        )
                        s4s.append(s4)
                    return s4s

                def norm_phase1b(pend, s4s):
                    po, ht, hb, q0 = pend
                    invs = []
                    for j in range(2):
                        inv4 = smalls.tile([P, 4], FP, tag="inv4")
                        nc.vector.reciprocal(inv4[:], s4s[j][:])
                        invd = invdram.tile([1, 512], FP, tag="invd")
                        nc.gpsimd.dma_start(
                            invd.rearrange("a (p f) -> (a p) f", p=P), inv4[:]
                        )
                        inv64 = smalls.tile([D, 512], FP, tag="inv64")
                        nc.gpsimd.dma_start(
                            inv64[:], invd[0:1, :].to_broadcast((D, 512))
                        )
                        invs.append(inv64)
                    return invs

                def norm_phase2(pend, invs):
                    po, ht, hb, q0 = pend
                    for j in range(2):
                        nc.vector.tensor_mul(
                            outT[ht][hb : hb + D, q0 + 512 * j : q0 + 512 * (j + 1)],
                            po[j][0:D, :],
                            invs[j][:],
                        )

                pending = None
                pending_s4 = None
                pending_invs = None
                for h in range(HL):
                    hb = (D * h) % P
                    ht = (D * h) // P
                    for half in range(2):
                        q0 = HQ * half
                        po = [
                            o_psum.tile([P, 512], FP, tag="o", name=f"po{h}_{half}_{j}")
                            for j in range(2)
                        ]
                        for i in range(NKT):
                            pss = []
                            ess = []
                            for j in range(2):
                                ps = s_psum.tile([P, 512], FP, tag="s")
                                nc.tensor.matmul(
                                    ps[:],
                                    (kT[ht][hb : hb + D, P * i : P * (i + 1)]),
                                    (qT[ht][hb : hb + D, q0 + 512 * j : q0 + 512 * (j + 1)]),
                                    start=True,
                                    stop=True,
                                )
                                pss.append(ps)
                            for j in range(2):
                                es = expst.tile([P, 512], F16, tag="e")
                                # 3 of every 5 chunks on ScalarE, 2 on DVE
                                if eidx % 5 not in (1, 3):
                                    nc.scalar.activation(
                                        es[:], pss[j][:], Exp, scale=SCALE
                                    )
                                else:
                                    nc.vector.tensor_scalar(
                                        es[:].bitcast(I16),
                                        pss[j][:],
                                        SCHRAUD_A,
                                        SCHRAUD_B,
                                        Mult,
                                        Add,
                                    )
                                eidx += 1
                                ess.append(es)
                            for j in range(2):
                                nc.tensor.matmul(
                                    po[j][:DV, :],
                                    vp[i][:, DV * h : DV * (h + 1)],
                                    ess[j][:],
                                    start=(i == 0),
                                    stop=(i == NKT - 1),
                                )
                            if pending is not None:
                                # phase spacing tracks the DMA completion
                                # latencies (~4.5us per 2-DMA bounce): each
                                # phase executes with its inputs already in
                                # SBUF, so no DVE op ever waits in-queue
                                if i == 0:
                                    pending_s4 = norm_phase1a(pending)
                                elif i == 5:
                                    pending_invs = norm_phase1b(pending, pending_s4)
                                elif i == 12:
                                    norm_phase2(pending, pending_invs)
                                    pending = None
                                    pending_s4 = None
                                    pending_invs = None
                        pending = (po, ht, hb, q0)
                # tail: normalize the final block
                pending_s4 = norm_phase1a(pending)
                pending_invs = norm_phase1b(pending, pending_s4)
                norm_phase2(pending, pending_invs)

            # ---------------- Stage C: partial output projection + bias -----
            with tc.tile_pool(name="f_psum", bufs=2, space="PSUM") as f_psum, \
                 tc.tile_pool(name="osb", bufs=4) as osb:
                for t in range(NST):
                    pf1 = f_psum.tile([P, 512], FP, tag="f1")
                    pf2 = f_psum.tile([P, 512], FP, tag="f2")
                    for c in range(FT):
                        nc.tensor.matmul(
                            pf1[:],
                            (outT[c][:, P * t : P * (t + 1)]),
                            (wproj_sb[c][:, 0:512]),
                            start=(c == 0),
                            stop=(c == FT - 1),
                        )
                    for c in range(FT):
                        nc.tensor.matmul(
                            pf2[:, :256],
                            (outT[c][:, P * t : P * (t + 1)]),
                            (wproj_sb[c][:, 512:E]),
                            start=(c == 0),
                            stop=(c == FT - 1),
                        )
                    # bias is added on the host during unsharding; stage C
                    # only evacuates (ScalarE, idle here) and stores fp16
                    ot = osb.tile([P, E], F16, tag="ot")
                    nc.scalar.copy(ot[:, 0:512], pf1[:])
                    nc.scalar.copy(ot[:, 512:E], pf2[:, :256])
                    nc.sync.dma_start(out[P * t : P * (t + 1), :], ot[:])

    _split_waits(nc)
    return nc


_CACHE = {}


def _get_nc():
    if "nc" not in _CACHE:
        _CACHE["nc"] = build()
    return _CACHE["nc"]


def make_in_maps(x, w_qkv, w_proj, b_proj):
    x = np.asarray(x, dtype=np.float32)
    w_qkv = np.asarray(w_qkv, np.float32)
    w_proj = np.asarray(w_proj, np.float32)
    b_proj = np.asarray(b_proj, np.float32)
    in_maps = []
    for c in range(NCORES):
        b, half = c // 2, c % 2
        heads = range(HL * half, HL * half + HL)
        rows = (
            [E * 0 + D * h + d for h in heads for d in range(D)]
            + [E * 1 + D * h + d for h in heads for d in range(D)]
            + [E * 2 + D * h + d for h in heads for d in range(D)]
        )
        wqkvT_l = np.ascontiguousarray(w_qkv[rows, :].T).astype(np.float16)
        wprojT_l = np.ascontiguousarray(w_proj[:, rows[: FL]].T).astype(np.float16)
        in_maps.append(
            {
                "identd": np.eye(P, dtype=np.float16),
                "xb": np.ascontiguousarray(x[b]).astype(np.float16),
                "wqkvT": wqkvT_l,
                "wprojT": wprojT_l,
            }
        )
    return in_maps


def assemble(results, b_proj):
    b_proj = np.asarray(b_proj, np.float32)
    outp = np.empty((B, S, E), np.float32)
    for b in range(B):
        outp[b] = (
            results[2 * b]["out"].astype(np.float32)
            + results[2 * b + 1]["out"].astype(np.float32)
            + b_proj
        )
    return outp


def kernel(x, w_qkv, w_proj, b_proj):
    nc = _get_nc()
    in_maps = make_in_maps(x, w_qkv, w_proj, b_proj)
    res = run_bass_kernel_spmd(nc, in_maps, core_ids=list(range(NCORES)))
    return assemble(res.results, b_proj)
